# revision 8
# baseline (speedup 1.0000x reference)
"""Trainium2 Bass kernel for nn_Attention_48799418417201.

Multi-head attention (B=8, S=1024, E=768, H=12, D=64) with LoRA (R=16) on the
QKV projections. Data-parallel over batch: one batch element per NeuronCore,
8 cores.

Key design points (v2):
  - LoRA is folded into the projection weights on the HOST (W_eff = W + B@A),
    eliminating all on-device LoRA matmuls (~10% of TensorE work).
  - Host passes x^T [E, S] per input (q/k/v) plus pre-transposed weights, all
    fp16. The 1/sqrt(D) scaling is folded into Wq/bq on the host.
  - Projections produce Q^T, K^T [E, S] (head-major partitions: head 2t at
    partitions 0-63 of tile t, head 2t+1 at 64-127) and V_aug [S, 13*65]
    (65 columns per head: 64 V columns + a ones column).
  - Scores are computed transposed: S^T[j, i] = sum_d K^T[d,j] Q^T[d,i].
    The two heads of a pair use DISJOINT partition ranges (0-63 / 64-127), so
    their matmuls land on different PE row-tiles (64x128 mode, T0/T8) and are
    emitted back-to-back into one 4-bank PSUM tile [128, 2, 1024] -> the PE
    runs them CONCURRENTLY (per-subarray row-tile concurrency), doubling
    effective throughput of the d=64-contraction scores.
  - One fused exp over [128, 2x1024] per (t, j) on ScalarE (the +352-cycle
    per-activation overhead is paid half as often).
  - PV runs j-outer with 4 held PSUM accumulators (hh x i) so each Va
    stationary load serves two consecutive matmuls; the ones column in V_aug
    makes the PV matmul emit the softmax denominator Z into PSUM row 64 free.
  - PV produces O^T [E, S] directly, which is the layout the output
    projection needs as its stationary side; no on-device transposes at all.
  - Pipeline: scores for pairs 0/1 are hooked onto the K projection's last
    chunk; the pair loop then runs PV(t) against scores(t+2).
"""

import numpy as np
from contextlib import ExitStack

import concourse.bass as bass
import concourse.bacc as bacc
import concourse.tile as tile
from concourse import mybir
from concourse.bass_utils import run_bass_kernel_spmd

P = 128
S = 1024  # sequence length
E = 768  # embedding
H = 12  # heads
D = 64  # head dim
NT = E // P  # 6 n-tiles (also e-tiles) per 768-wide dim
MC = S // 512  # 2 moving-chunks of 512 along sequence
MS = S // P  # 8 sequence subtiles of 128
JT = S // P  # 8 j-tiles (key blocks)
IC = S // 512  # 2 i-chunks (query blocks of 512)
VW = D + 1  # 65 columns per head in V_aug

F16 = mybir.dt.float16
F32 = mybir.dt.float32


def build_nc():
    nc = bacc.Bacc("TRN2", target_bir_lowering=False, debug=False, num_devices=8)

    xT = {
        name: nc.dram_tensor(f"x{name}T", [E, S], F16, kind="ExternalInput")
        for name in ("q", "k", "v")
    }
    wT_d = nc.dram_tensor("wT", [E, 3 * E], F16, kind="ExternalInput")
    woT_d = nc.dram_tensor("woT", [E, E], F16, kind="ExternalInput")
    bqk_d = nc.dram_tensor("bqk", [P, 2 * NT], F32, kind="ExternalInput")
    bv_d = nc.dram_tensor("bv", [E], F32, kind="ExternalInput")
    ob_d = nc.dram_tensor("ob", [E], F32, kind="ExternalInput")
    out_d = nc.dram_tensor("out", [S, E], F32, kind="ExternalOutput")

    with tile.TileContext(nc) as tc, ExitStack() as perm:
        pp = perm.enter_context(tc.tile_pool(name="perm", bufs=1))

        QT = [pp.tile([P, S], F16, name=f"QT{t}", tag=f"QT{t}") for t in range(NT)]
        KT = [pp.tile([P, S], F16, name=f"KT{t}", tag=f"KT{t}") for t in range(NT)]
        Va = [pp.tile([P, H * VW], F16, name=f"Va{m}", tag=f"Va{m}") for m in range(MS)]
        woT = [pp.tile([P, E], F16, name=f"woT{t}", tag=f"woT{t}") for t in range(NT)]
        bqk = pp.tile([P, 2 * NT], F32, name="bqk", tag="bqk")
        bv_sb = pp.tile([P, E], F32, name="bv_sb", tag="bv_sb")
        ob_sb = pp.tile([P, E], F32, name="ob_sb", tag="ob_sb")
        zbias = pp.tile([P, 1], F32, name="zbias", tag="zbias")

        nc.vector.memset(zbias[:], 0.0)
        nc.sync.dma_start(bqk[:], bqk_d.ap()[:])

        outacc = [
            pp.tile([P, E], F32, name=f"outacc{m}", tag=f"outacc{m}")
            for m in range(MS)
        ]

        # ---------------- pools ----------------
        # PSUM budget (8 banks): stp (1x4 banks, the (j, hh-pair) scores tile)
        # + ppsum 3 during projections = 7; after the V projection ppsum is
        # released and pvp (2x1, the (i0, i1) PV accumulators of the current
        # hh) plus outp (2x1, fused out-proj psum) take its place: 4+2+2 = 8.
        # SBUF pool stack order matters: wqk (after K proj), wpv and xp (after
        # V proj) release LIFO, funding ep2.
        stp = tc.alloc_tile_pool(name="stp", bufs=1, space="PSUM")
        ppsum = tc.alloc_tile_pool(name="ppsum", bufs=3, space="PSUM")
        ep = tc.alloc_tile_pool(name="ep", bufs=14)
        otup = tc.alloc_tile_pool(name="otup", bufs=2)
        sgp = tc.alloc_tile_pool(name="sgp", bufs=2)
        zbp = tc.alloc_tile_pool(name="zbp", bufs=2)
        zsp = tc.alloc_tile_pool(name="zsp", bufs=2)
        dpool = tc.alloc_tile_pool(name="dpool", bufs=1, space="DRAM")
        xp = tc.alloc_tile_pool(name="xp", bufs=2)
        wpv = tc.alloc_tile_pool(name="wpv", bufs=1)
        wqk = tc.alloc_tile_pool(name="wqk", bufs=1)
        zdram = dpool.tile([H, S], F32, name="zdram", tag="zdram")

        wreg = {}
        for name in ("q", "k"):
            wreg[name] = [
                wqk.tile([P, E], F16, name=f"w{name}{k}", tag=f"w{name}{k}")
                for k in range(NT)
            ]
        wreg["v"] = [
            wpv.tile([P, E], F16, name=f"wv{k}", tag=f"wv{k}") for k in range(NT)
        ]

        def emit_proj_qk(name, after_n=None):
            noff = (0 if name == "q" else E)
            dest = QT if name == "q" else KT
            bcol = 0 if name == "q" else NT
            for m in range(MC):
                msl = slice(m * 512, (m + 1) * 512)
                xc = xp.tile([P, NT, 512], F16, name=f"xc_{name}{m}", tag="xc")
                for k in range(NT):
                    nc.sync.dma_start(
                        xc[:, k, :], xT[name].ap()[k * P : (k + 1) * P, msl]
                    )
                if m == 0:
                    for k in range(NT):
                        nc.sync.dma_start(
                            wreg[name][k][:],
                            wT_d.ap()[k * P : (k + 1) * P, noff : noff + E],
                        )
                for n in range(NT):
                    nsl = slice(n * P, (n + 1) * P)
                    acc = ppsum.tile([P, 512], F32, name=f"acc_{name}{m}_{n}", tag="acc")
                    for k in range(NT):
                        nc.tensor.matmul(
                            acc[:], wreg[name][k][:, nsl], xc[:, k, :],
                            start=(k == 0), stop=(k == NT - 1),
                        )
                    nc.vector.tensor_scalar_add(
                        dest[n][:, msl], acc[:], bqk[:, bcol + n : bcol + n + 1]
                    )
                    if after_n is not None and m == MC - 1:
                        after_n(n)

        def emit_v_setup():
            nc.sync.dma_start(bv_sb[:], bv_d.ap().partition_broadcast(P))
            for g in range(MS):
                va_cols = Va[g].rearrange("p (h c) -> p h c", c=VW)
                nc.vector.memset(va_cols[:, :, D], 1.0)

        def emit_proj_v(m):
            noff = 2 * E
            msl = slice(m * 512, (m + 1) * 512)
            xc = xp.tile([P, NT, 512], F16, name=f"xc_v{m}", tag="xc")
            for k in range(NT):
                nc.sync.dma_start(
                    xc[:, k, :], xT["v"].ap()[k * P : (k + 1) * P, msl]
                )
            if m == 0:
                for k in range(NT):
                    nc.sync.dma_start(
                        wreg["v"][k][:],
                        wT_d.ap()[k * P : (k + 1) * P, noff : noff + E],
                    )
            for ms_i in range(4):
                g = m * 4 + ms_i
                for nch in range(2):
                    ncols = 512 if nch == 0 else E - 512
                    nsl = slice(nch * 512, nch * 512 + ncols)
                    acc = ppsum.tile([P, 512], F32, name=f"accv{g}_{nch}", tag="acc")
                    for k in range(NT):
                        nc.tensor.matmul(
                            acc[:, :ncols],
                            xc[:, k, ms_i * P : (ms_i + 1) * P],
                            wreg["v"][k][:, nsl],
                            start=(k == 0), stop=(k == NT - 1),
                        )
                    h0 = nch * 8
                    nh = 8 if nch == 0 else 4
                    for hi in range(nh):
                        h = h0 + hi
                        nc.vector.tensor_add(
                            Va[g][:, h * VW : h * VW + D],
                            acc[:, h * D - nch * 512 : (h + 1) * D - nch * 512],
                            bv_sb[:, h * D : (h + 1) * D],
                        )

        exps = {}

        def emit_scores(t):
            # Head pair (2t, 2t+1) lives at partitions 0-63 / 64-127 of
            # QT[t]/KT[t].  The two heads' score matmuls use disjoint PE
            # row-tiles (tile_size 64x128 at row 0 / row 64) and disjoint
            # PSUM banks of one [128, 2, 1024] tile, so the PE overlaps them.
            for j in range(JT):
                jsl = slice(j * P, (j + 1) * P)
                st = stp.tile([P, 2, S], F32, name=f"st{t}_{j}", tag="st")
                for i in range(IC):
                    isl = slice(i * 512, (i + 1) * 512)
                    nc.tensor.matmul(
                        st[:, 0, isl],
                        KT[t][0:D, jsl],
                        QT[t][0:D, isl],
                    )
                    nc.tensor.matmul(
                        st[:, 1, isl],
                        KT[t][D : 2 * D, jsl],
                        QT[t][D : 2 * D, isl],
                    )
                pool = ep2 if (t >= 2 and j < 3) else ep
                ex = pool.tile([P, 2, S], F16, name=f"ex{t}_{j}", tag="ex")
                nc.scalar.activation(
                    ex[:], st[:], mybir.ActivationFunctionType.Exp, bias=zbias[:]
                )
                exps[(t, j)] = ex

        ot = {}

        def emit_pv(t):
            ot[t] = (
                otup.tile([P, S], F16, name=f"OTu{t}", tag="OTu"),
                otup.tile([P, S], F16, name=f"OTn{t}", tag="OTn"),
            )
            OTu_t, OTn_t = ot[t]
            # hh-outer with 2 held accumulators (i0, i1): each Va stationary
            # load (per (hh, j)) serves the two i-chunk matmuls back to back,
            # and only 2 PSUM banks are held (the other 2 feed the fused
            # output projection).
            zt = {}
            for hh in range(2):
                h = 2 * t + hh
                base = hh * D
                pv = [
                    pvp.tile([P, 512], F32, name=f"pv{t}_{hh}_{i}", tag="pv")
                    for i in range(IC)
                ]
                for j in range(JT):
                    for i in range(IC):
                        isl = slice(i * 512, (i + 1) * 512)
                        nc.tensor.matmul(
                            pv[i][0:VW, :],
                            Va[j][:, h * VW : (h + 1) * VW],
                            exps[(t, j)][:, hh, isl],
                            start=(j == 0), stop=(j == JT - 1),
                        )
                for i in range(IC):
                    isl = slice(i * 512, (i + 1) * 512)
                    if (t, i) not in zt:
                        zt[(t, i)] = zsp.tile([2, 512], F16, name=f"zt{t}_{i}", tag="zt")
                    stage = sgp.tile([VW, 512], F16, name=f"stg{t}_{hh}_{i}", tag="stg")
                    nc.vector.tensor_copy(stage[:], pv[i][0:VW, :])
                    nc.sync.dma_start(OTu_t[base : base + D, isl], stage[0:D, :])
                    nc.sync.dma_start(zt[(t, i)][hh : hh + 1, :], stage[D : D + 1, :])
            zb = zbp.tile([P, S], F32, name=f"zb{t}", tag="zb")
            for i in range(IC):
                isl = slice(i * 512, (i + 1) * 512)
                z32 = zsp.tile([2, 512], F32, name=f"z32_{t}_{i}", tag="z32")
                rz = zsp.tile([2, 512], F32, name=f"rz{t}_{i}", tag="rz")
                nc.vector.tensor_copy(z32[:], zt[(t, i)][:])
                nc.vector.reciprocal_approx_fast(rz[:], z32[:])
                nc.sync.dma_start(zdram[2 * t : 2 * t + 2, isl], rz[:])
                for hh in range(2):
                    nc.sync.dma_start(
                        zb[hh * D : (hh + 1) * D, isl],
                        zdram[2 * t + hh, isl].partition_broadcast(D),
                    )
                nc.vector.tensor_mul(OTn_t[:, isl], OTu_t[:, isl], zb[:, isl])

        def emit_outproj(t):
            # Fused output projection: right after OTn[t] is normalized, its
            # contribution OTn[t]^T @ woT[t] is accumulated into the SBUF
            # accumulators (outacc[m] was pre-initialized with the bias), so
            # no serial out-proj tail remains after the last PV.
            for m in range(MS):
                for nch in range(2):
                    ncols = 512 if nch == 0 else E - 512
                    nsl = slice(nch * 512, nch * 512 + ncols)
                    acc = outp.tile([P, 512], F32, name=f"oa{t}_{m}_{nch}", tag="oacc")
                    nc.tensor.matmul(
                        acc[:, :ncols],
                        ot[t][1][:, m * P : (m + 1) * P],
                        woT[t][:, nsl],
                    )
                    nc.vector.tensor_add(
                        outacc[m][:, nsl], outacc[m][:, nsl], acc[:, :ncols]
                    )

        # ---------------- emission sequence ----------------
        emit_proj_qk("q")
        ep2 = None

        def fire_early(n):
            if n < 2:
                emit_scores(n)

        emit_proj_qk("k", after_n=fire_early)
        # wq/wk are dead after the k projection; freeing them funds extra expS
        # slots so pair-2+ scores can queue while earlier pairs are consumed.
        wqk.release()
        emit_v_setup()
        emit_proj_v(0)
        emit_proj_v(1)
        wpv.release()
        xp.release()
        ep2 = tc.alloc_tile_pool(name="ep2", bufs=5)
        for t in range(NT):
            nc.sync.dma_start(woT[t][:], woT_d.ap()[t * P : (t + 1) * P, :])
        nc.sync.dma_start(ob_sb[:], ob_d.ap().partition_broadcast(P))
        for m in range(MS):
            nc.vector.tensor_copy(outacc[m][:], ob_sb[:])
        # the projection accumulators are dead; their 3 PSUM banks plus the
        # spare bank become the PV accumulators and the out-proj psum.
        ppsum.release()
        pvp = tc.alloc_tile_pool(name="pvp", bufs=2, space="PSUM")
        outp = tc.alloc_tile_pool(name="outp", bufs=2, space="PSUM")
        for t in range(NT):
            emit_pv(t)
            emit_outproj(t)
            if t + 2 < NT:
                emit_scores(t + 2)
        for m in range(MS):
            nc.sync.dma_start(out_d.ap()[m * P : (m + 1) * P, :], outacc[m][:])
        ep2.release()
        zsp.release()
        zbp.release()
        sgp.release()
        otup.release()
        ep.release()
        dpool.release()
        outp.release()
        pvp.release()
        stp.release()

    nc.compile()
    return nc


def _prep_inputs(q, k, v, in_proj_weight, in_proj_bias, out_w, out_b, lora_a, lora_b):
    scale = float(D) ** -0.5
    q = np.asarray(q, np.float32)
    k = np.asarray(k, np.float32)
    v = np.asarray(v, np.float32)
    in_proj_weight = np.asarray(in_proj_weight, np.float32)
    in_proj_bias = np.asarray(in_proj_bias, np.float32)
    out_w = np.asarray(out_w, np.float32)
    out_b = np.asarray(out_b, np.float32)
    lora_a = np.asarray(lora_a, np.float32)
    lora_b = np.asarray(lora_b, np.float32)

    # fold the LoRA delta into the projection weights
    w_eff = in_proj_weight + lora_b @ lora_a  # [3E, E]
    wT = w_eff.T.copy()  # [E, 3E]
    wT[:, :E] *= scale
    bq = (in_proj_bias[:E] * scale).reshape(NT, P).T  # [P, NT]
    bk = in_proj_bias[E : 2 * E].reshape(NT, P).T
    bqk = np.ascontiguousarray(np.concatenate([bq, bk], axis=1), np.float32)

    shared = {
        "wT": np.ascontiguousarray(wT, np.float16),
        "woT": np.ascontiguousarray(out_w.T, np.float16),
        "bqk": bqk,
        "bv": np.ascontiguousarray(in_proj_bias[2 * E :], np.float32),
        "ob": np.ascontiguousarray(out_b, np.float32),
    }
    in_maps = []
    for b in range(8):
        m = dict(shared)
        m["xqT"] = np.ascontiguousarray(q[b].T, np.float16)
        m["xkT"] = np.ascontiguousarray(k[b].T, np.float16)
        m["xvT"] = np.ascontiguousarray(v[b].T, np.float16)
        in_maps.append(m)
    return in_maps


_NC_CACHE = {}


def run(inputs, trace=False, **spmd_kwargs):
    if "nc" not in _NC_CACHE:
        _NC_CACHE["nc"] = build_nc()
    nc = _NC_CACHE["nc"]
    in_maps = _prep_inputs(
        inputs["q"],
        inputs["k"],
        inputs["v"],
        inputs["in_proj_weight"],
        inputs["in_proj_bias"],
        inputs["out_w"],
        inputs["out_b"],
        inputs["lora_a"],
        inputs["lora_b"],
    )
    res = run_bass_kernel_spmd(
        nc, in_maps, core_ids=list(range(8)), trace=trace, **spmd_kwargs
    )
    out = np.stack([res.results[b]["out"] for b in range(8)]).astype(np.float32)
    return out, res


def kernel(
    q,
    k,
    v,
    in_proj_weight,
    in_proj_bias,
    out_w,
    out_b,
    lora_a,
    lora_b,
    num_heads=12,
    **_unused,
):
    assert int(num_heads) == H
    out, _ = run(
        {
            "q": q,
            "k": k,
            "v": v,
            "in_proj_weight": in_proj_weight,
            "in_proj_bias": in_proj_bias,
            "out_w": out_w,
            "out_b": out_b,
            "lora_a": lora_a,
            "lora_b": lora_b,
        }
    )
    return out


# revision 10
# speedup vs baseline: 1.1855x; 1.1855x over previous
"""Trainium2 Bass kernel for nn_Attention_48799418417201.

Multi-head attention (B=8, S=1024, E=768, H=12, D=64) with LoRA (R=16) on the
QKV projections. Data-parallel over batch: one batch element per NeuronCore,
8 cores.

Key design points (v3):
  - LoRA is folded into the projection weights on the HOST (W_eff = W + B@A),
    eliminating all on-device LoRA matmuls (~10% of TensorE work).
  - Host passes x^T [E, S] per input (q/k/v) plus pre-transposed weights, all
    fp16. The 1/sqrt(D) scaling is folded into Wq/bq on the host.
  - Q/K projections are emitted n-outer and INTERLEAVED (q(n), k(n)), so
    QT[0]/KT[0] are complete a few microseconds in and the exp pipeline (the
    rate limiter) starts almost immediately instead of after both full
    projections.
  - Scores are computed transposed: S^T[j, i] = sum_d K^T[d,j] Q^T[d,i].
    The two heads of a pair use DISJOINT partition ranges (0-63 / 64-127), so
    their matmuls land on different PE row-tiles (64x128 mode, T0/T8) and are
    emitted back-to-back into one 4-bank PSUM tile [128, 2, 1024] -> the PE
    runs them CONCURRENTLY (per-subarray row-tile concurrency), doubling
    effective throughput of the d=64-contraction scores.
  - One fused exp over [128, 2x1024] per (t, j) on ScalarE (the per-activation
    fixed overhead is paid half as often).
  - PV runs j-outer with 4 held PSUM accumulators (hh x i) so each Va
    stationary load serves two consecutive matmuls; the ones column in V_aug
    makes the PV matmul emit the softmax denominator Z into PSUM row 64 free.
  - PV produces O^T [E, S] directly, which is the layout the output
    projection needs as its stationary side; no on-device transposes at all.
  - Pipeline: pv(t) runs against scores(t+2); remaining projection work and
    PV fill TensorE while ScalarE chews exps.
"""

import numpy as np
from contextlib import ExitStack

import concourse.bass as bass
import concourse.bacc as bacc
import concourse.tile as tile
from concourse import mybir
from concourse.bass_utils import run_bass_kernel_spmd

P = 128
S = 1024  # sequence length
E = 768  # embedding
H = 12  # heads
D = 64  # head dim
NT = E // P  # 6 n-tiles (also e-tiles) per 768-wide dim
MC = S // 512  # 2 moving-chunks of 512 along sequence
MS = S // P  # 8 sequence subtiles of 128
JT = S // P  # 8 j-tiles (key blocks)
IC = S // 512  # 2 i-chunks (query blocks of 512)
VW = D + 1  # 65 columns per head in V_aug

F16 = mybir.dt.float16
F32 = mybir.dt.float32


def build_nc():
    nc = bacc.Bacc("TRN2", target_bir_lowering=False, debug=False, num_devices=8)

    xT = {
        name: nc.dram_tensor(f"x{name}T", [E, S], F16, kind="ExternalInput")
        for name in ("q", "k", "v")
    }
    wT_d = nc.dram_tensor("wT", [E, 3 * E], F16, kind="ExternalInput")
    woT_d = nc.dram_tensor("woT", [E, E], F16, kind="ExternalInput")
    bqk_d = nc.dram_tensor("bqk", [P, 2 * NT], F32, kind="ExternalInput")
    bv_d = nc.dram_tensor("bv", [E], F32, kind="ExternalInput")
    ob_d = nc.dram_tensor("ob", [E], F32, kind="ExternalInput")
    out_d = nc.dram_tensor("out", [S, E], F32, kind="ExternalOutput")

    with tile.TileContext(nc) as tc, ExitStack() as perm:
        pp = perm.enter_context(tc.tile_pool(name="perm", bufs=1))

        QT = [pp.tile([P, S], F16, name=f"QT{t}", tag=f"QT{t}") for t in range(NT)]
        KT = [pp.tile([P, S], F16, name=f"KT{t}", tag=f"KT{t}") for t in range(NT)]
        Va = [pp.tile([P, H * VW], F16, name=f"Va{m}", tag=f"Va{m}") for m in range(MS)]
        woT = [pp.tile([P, E], F16, name=f"woT{t}", tag=f"woT{t}") for t in range(NT)]
        OTn = [pp.tile([P, S], F16, name=f"OTn{t}", tag=f"OTn{t}") for t in range(NT)]
        bqk = pp.tile([P, 2 * NT], F32, name="bqk", tag="bqk")
        bv_sb = pp.tile([P, E], F32, name="bv_sb", tag="bv_sb")
        ob_sb = pp.tile([P, E], F32, name="ob_sb", tag="ob_sb")
        zbias = pp.tile([P, 1], F32, name="zbias", tag="zbias")

        nc.vector.memset(zbias[:], 0.0)
        nc.sync.dma_start(bqk[:], bqk_d.ap()[:])

        # ---------------- pools ----------------
        # PSUM budget (8 banks): stp (1x4 banks, the (j, hh-pair) scores tile)
        # + ppsum 3 during projections = 7; after the V projection ppsum is
        # released and pvp (4x1, the (hh, i) PV accumulators) takes its place.
        # SBUF pool stack order matters: wqk (after K proj), wpv and xp (after
        # V proj) release LIFO, funding ep2.
        stp = tc.alloc_tile_pool(name="stp", bufs=1, space="PSUM")
        ppsum = tc.alloc_tile_pool(name="ppsum", bufs=3, space="PSUM")
        ep = tc.alloc_tile_pool(name="ep", bufs=12)
        otup = tc.alloc_tile_pool(name="otup", bufs=2)
        sgp = tc.alloc_tile_pool(name="sgp", bufs=2)
        zbp = tc.alloc_tile_pool(name="zbp", bufs=2)
        zsp = tc.alloc_tile_pool(name="zsp", bufs=2)
        dpool = tc.alloc_tile_pool(name="dpool", bufs=1, space="DRAM")
        xp = tc.alloc_tile_pool(name="xp", bufs=4)
        wpv = tc.alloc_tile_pool(name="wpv", bufs=1)
        wqk = tc.alloc_tile_pool(name="wqk", bufs=1)
        zdram = dpool.tile([H, S], F32, name="zdram", tag="zdram")

        wreg = {}
        for name in ("q", "k"):
            wreg[name] = [
                wqk.tile([P, E], F16, name=f"w{name}{k}", tag=f"w{name}{k}")
                for k in range(NT)
            ]
        wreg["v"] = [
            wpv.tile([P, E], F16, name=f"wv{k}", tag=f"wv{k}") for k in range(NT)
        ]

        exps = {}

        def emit_scores(t):
            # Head pair (2t, 2t+1) lives at partitions 0-63 / 64-127 of
            # QT[t]/KT[t].  The two heads' score matmuls use disjoint PE
            # row-tiles (tile_size 64x128 at row 0 / row 64) and disjoint
            # PSUM banks of one [128, 2, 1024] tile, so the PE overlaps them.
            for j in range(JT):
                jsl = slice(j * P, (j + 1) * P)
                st = stp.tile([P, 2, S], F32, name=f"st{t}_{j}", tag="st")
                for i in range(IC):
                    isl = slice(i * 512, (i + 1) * 512)
                    nc.tensor.matmul(
                        st[:, 0, isl],
                        KT[t][0:D, jsl],
                        QT[t][0:D, isl],
                    )
                    nc.tensor.matmul(
                        st[:, 1, isl],
                        KT[t][D : 2 * D, jsl],
                        QT[t][D : 2 * D, isl],
                    )
                pool = ep2 if (t >= 2 and j < 3) else ep
                ex = pool.tile([P, 2, S], F16, name=f"ex{t}_{j}", tag="ex")
                nc.scalar.activation(
                    ex[:], st[:], mybir.ActivationFunctionType.Exp, bias=zbias[:]
                )
                exps[(t, j)] = ex

        def emit_proj_qk_interleaved():
            # All four x chunks (q/k x m0/m1) live at once (xp bufs=4);
            # n-outer emission completes QT[n]/KT[n] together so scores(n)
            # can fire immediately for n < 2.
            xc = {}
            for name in ("q", "k"):
                noff = 0 if name == "q" else E
                for k in range(NT):
                    nc.sync.dma_start(
                        wreg[name][k][:],
                        wT_d.ap()[k * P : (k + 1) * P, noff : noff + E],
                    )
                for m in range(MC):
                    msl = slice(m * 512, (m + 1) * 512)
                    t_ = xp.tile([P, NT, 512], F16, name=f"xc_{name}{m}", tag="xc")
                    for k in range(NT):
                        nc.sync.dma_start(
                            t_[:, k, :], xT[name].ap()[k * P : (k + 1) * P, msl]
                        )
                    xc[(name, m)] = t_
            for n in range(NT):
                nsl = slice(n * P, (n + 1) * P)
                for name, dest, bcol in (("q", QT, 0), ("k", KT, NT)):
                    for m in range(MC):
                        msl = slice(m * 512, (m + 1) * 512)
                        acc = ppsum.tile(
                            [P, 512], F32, name=f"acc_{name}{m}_{n}", tag="acc"
                        )
                        for k in range(NT):
                            nc.tensor.matmul(
                                acc[:], wreg[name][k][:, nsl], xc[(name, m)][:, k, :],
                                start=(k == 0), stop=(k == NT - 1),
                            )
                        nc.vector.tensor_scalar_add(
                            dest[n][:, msl], acc[:], bqk[:, bcol + n : bcol + n + 1]
                        )
                if n < 2:
                    emit_scores(n)

        def emit_v_setup():
            nc.sync.dma_start(bv_sb[:], bv_d.ap().partition_broadcast(P))
            for g in range(MS):
                va_cols = Va[g].rearrange("p (h c) -> p h c", c=VW)
                nc.vector.memset(va_cols[:, :, D], 1.0)

        def emit_proj_v(m):
            noff = 2 * E
            msl = slice(m * 512, (m + 1) * 512)
            xc = xp.tile([P, NT, 512], F16, name=f"xc_v{m}", tag="xc")
            for k in range(NT):
                nc.sync.dma_start(
                    xc[:, k, :], xT["v"].ap()[k * P : (k + 1) * P, msl]
                )
            if m == 0:
                for k in range(NT):
                    nc.sync.dma_start(
                        wreg["v"][k][:],
                        wT_d.ap()[k * P : (k + 1) * P, noff : noff + E],
                    )
            for ms_i in range(4):
                g = m * 4 + ms_i
                for nch in range(2):
                    ncols = 512 if nch == 0 else E - 512
                    nsl = slice(nch * 512, nch * 512 + ncols)
                    acc = ppsum.tile([P, 512], F32, name=f"accv{g}_{nch}", tag="acc")
                    for k in range(NT):
                        nc.tensor.matmul(
                            acc[:, :ncols],
                            xc[:, k, ms_i * P : (ms_i + 1) * P],
                            wreg["v"][k][:, nsl],
                            start=(k == 0), stop=(k == NT - 1),
                        )
                    h0 = nch * 8
                    nh = 8 if nch == 0 else 4
                    for hi in range(nh):
                        h = h0 + hi
                        nc.vector.tensor_add(
                            Va[g][:, h * VW : h * VW + D],
                            acc[:, h * D - nch * 512 : (h + 1) * D - nch * 512],
                            bv_sb[:, h * D : (h + 1) * D],
                        )

        def emit_pv(t):
            # j-outer with 4 held accumulators: each Va stationary load (per
            # (j, hh)) serves the two i-chunk matmuls back to back.
            OTu_t = otup.tile([P, S], F16, name=f"OTu{t}", tag="OTu")
            OTn_t = OTn[t]
            pv = {}
            for hh in range(2):
                for i in range(IC):
                    pv[(hh, i)] = pvp.tile(
                        [P, 512], F32, name=f"pv{t}_{hh}_{i}", tag="pv"
                    )
            for j in range(JT):
                for hh in range(2):
                    h = 2 * t + hh
                    for i in range(IC):
                        isl = slice(i * 512, (i + 1) * 512)
                        nc.tensor.matmul(
                            pv[(hh, i)][0:VW, :],
                            Va[j][:, h * VW : (h + 1) * VW],
                            exps[(t, j)][:, hh, isl],
                            start=(j == 0), stop=(j == JT - 1),
                        )
            zb = zbp.tile([P, S], F32, name=f"zb{t}", tag="zb")
            for i in range(IC):
                isl = slice(i * 512, (i + 1) * 512)
                zt = zsp.tile([2, 512], F16, name=f"zt{t}_{i}", tag="zt")
                for hh in range(2):
                    base = hh * D
                    stage = sgp.tile([VW, 512], F16, name=f"stg{t}_{hh}_{i}", tag="stg")
                    nc.vector.tensor_copy(stage[:], pv[(hh, i)][0:VW, :])
                    nc.sync.dma_start(OTu_t[base : base + D, isl], stage[0:D, :])
                    nc.sync.dma_start(zt[hh : hh + 1, :], stage[D : D + 1, :])
                z32 = zsp.tile([2, 512], F32, name=f"z32_{t}_{i}", tag="z32")
                rz = zsp.tile([2, 512], F32, name=f"rz{t}_{i}", tag="rz")
                nc.vector.tensor_copy(z32[:], zt[:])
                nc.vector.reciprocal_approx_fast(rz[:], z32[:])
                nc.sync.dma_start(zdram[2 * t : 2 * t + 2, isl], rz[:])
                for hh in range(2):
                    nc.sync.dma_start(
                        zb[hh * D : (hh + 1) * D, isl],
                        zdram[2 * t + hh, isl].partition_broadcast(D),
                    )
                nc.vector.tensor_mul(OTn_t[:, isl], OTu_t[:, isl], zb[:, isl])

        # ---------------- emission sequence ----------------
        ep2 = None
        emit_proj_qk_interleaved()
        # wq/wk are dead after the k projection; freeing them funds extra expS
        # slots so pair-2+ scores can queue while earlier pairs are consumed.
        wqk.release()
        emit_v_setup()
        emit_proj_v(0)
        emit_proj_v(1)
        wpv.release()
        xp.release()
        ep2 = tc.alloc_tile_pool(name="ep2", bufs=5)
        for t in range(NT):
            nc.sync.dma_start(woT[t][:], woT_d.ap()[t * P : (t + 1) * P, :])
        # the projection accumulators are dead; their 3 PSUM banks plus the
        # spare bank become the 4 held PV accumulators.
        ppsum.release()
        pvp = tc.alloc_tile_pool(name="pvp", bufs=4, space="PSUM")
        for t in range(NT):
            emit_pv(t)
            if t + 2 < NT:
                emit_scores(t + 2)
        ep2.release()
        zsp.release()
        zbp.release()
        sgp.release()
        otup.release()
        ep.release()
        dpool.release()
        pvp.release()
        stp.release()

        # ---------------- Phase O: output projection ----------------
        with ExitStack() as octx:
            op = octx.enter_context(tc.tile_pool(name="op", bufs=4, space="PSUM"))
            fp = octx.enter_context(tc.tile_pool(name="fp", bufs=3))

            nc.sync.dma_start(ob_sb[:], ob_d.ap().partition_broadcast(P))
            for m in range(MS):
                acc = op.tile([P, S], F32, name=f"oacc{m}", tag="oacc")
                for e in range(NT):
                    for nch in range(2):
                        ncols = 512 if nch == 0 else E - 512
                        nsl = slice(nch * 512, nch * 512 + ncols)
                        nc.tensor.matmul(
                            acc[:, nsl],
                            OTn[e][:, m * P : (m + 1) * P],
                            woT[e][:, nsl],
                            start=(e == 0),
                            stop=(e == NT - 1),
                        )
                fin = fp.tile([P, E], F32, name=f"fin{m}", tag="fin")
                nc.vector.tensor_add(fin[:], acc[:, :E], ob_sb[:])
                nc.sync.dma_start(out_d.ap()[m * P : (m + 1) * P, :], fin[:])

    nc.compile()
    return nc


def _prep_inputs(q, k, v, in_proj_weight, in_proj_bias, out_w, out_b, lora_a, lora_b):
    scale = float(D) ** -0.5
    q = np.asarray(q, np.float32)
    k = np.asarray(k, np.float32)
    v = np.asarray(v, np.float32)
    in_proj_weight = np.asarray(in_proj_weight, np.float32)
    in_proj_bias = np.asarray(in_proj_bias, np.float32)
    out_w = np.asarray(out_w, np.float32)
    out_b = np.asarray(out_b, np.float32)
    lora_a = np.asarray(lora_a, np.float32)
    lora_b = np.asarray(lora_b, np.float32)

    # fold the LoRA delta into the projection weights
    w_eff = in_proj_weight + lora_b @ lora_a  # [3E, E]
    wT = w_eff.T.copy()  # [E, 3E]
    wT[:, :E] *= scale
    bq = (in_proj_bias[:E] * scale).reshape(NT, P).T  # [P, NT]
    bk = in_proj_bias[E : 2 * E].reshape(NT, P).T
    bqk = np.ascontiguousarray(np.concatenate([bq, bk], axis=1), np.float32)

    shared = {
        "wT": np.ascontiguousarray(wT, np.float16),
        "woT": np.ascontiguousarray(out_w.T, np.float16),
        "bqk": bqk,
        "bv": np.ascontiguousarray(in_proj_bias[2 * E :], np.float32),
        "ob": np.ascontiguousarray(out_b, np.float32),
    }
    in_maps = []
    for b in range(8):
        m = dict(shared)
        m["xqT"] = np.ascontiguousarray(q[b].T, np.float16)
        m["xkT"] = np.ascontiguousarray(k[b].T, np.float16)
        m["xvT"] = np.ascontiguousarray(v[b].T, np.float16)
        in_maps.append(m)
    return in_maps


_NC_CACHE = {}


def run(inputs, trace=False, **spmd_kwargs):
    if "nc" not in _NC_CACHE:
        _NC_CACHE["nc"] = build_nc()
    nc = _NC_CACHE["nc"]
    in_maps = _prep_inputs(
        inputs["q"],
        inputs["k"],
        inputs["v"],
        inputs["in_proj_weight"],
        inputs["in_proj_bias"],
        inputs["out_w"],
        inputs["out_b"],
        inputs["lora_a"],
        inputs["lora_b"],
    )
    res = run_bass_kernel_spmd(
        nc, in_maps, core_ids=list(range(8)), trace=trace, **spmd_kwargs
    )
    out = np.stack([res.results[b]["out"] for b in range(8)]).astype(np.float32)
    return out, res


def kernel(
    q,
    k,
    v,
    in_proj_weight,
    in_proj_bias,
    out_w,
    out_b,
    lora_a,
    lora_b,
    num_heads=12,
    **_unused,
):
    assert int(num_heads) == H
    out, _ = run(
        {
            "q": q,
            "k": k,
            "v": v,
            "in_proj_weight": in_proj_weight,
            "in_proj_bias": in_proj_bias,
            "out_w": out_w,
            "out_b": out_b,
            "lora_a": lora_a,
            "lora_b": lora_b,
        }
    )
    return out
